# revision 8
# baseline (speedup 1.0000x reference)
"""DualEncoder (two shared-weight LSTM encoders + bilinear score) on 8 trn2
NeuronCores — v3: two phase-offset chains (ctx / resp).

Sharding: 8-way tensor parallelism over the 4H gate dimension, computing
gates TRANSPOSED: per core, per chain, 4 PSUM tiles [128 gate-slice rows,
64 seq cols] (f/i/o/g). Stationary operands are weight tiles
(host-pretransposed, bf16), streaming operands are h^T / x^T tiles, so the
cell update emits h^T directly — no PE transposes, no h copy. x^T comes
straight from a transposing SWDGE dma_gather of the (host-cast bf16)
embedding table (one gather covers 2 steps x both chains).

v3 core idea: the ctx (seqs 0:64) and resp (seqs 64:128) encoders are
INDEPENDENT recurrences over the same weights. Running them as two
interleaved chains means the PE computes chain B's recurrent+proj matmuls
while chain A's h-slice broadcast is in flight (and vice versa): the
broadcast latency leaves the critical path, the PE stays continuously busy
with real work (no keep-warm dummies needed, HAM stays at K=8/8).

PSUM: single accumulation group per bank; chain A gates on banks 0-3,
chain B on banks 4-7. proj(t+1) for gate m re-opens the bank right after
ACT finished reading gates(t) (per-gate s_acts guard) so there is no
PE-write/ACT-read collision. Per-sender arrival semaphores (strict A,B
alternation per sender on queue 0 keeps counts ordered); own slice is
signaled locally by Pool. Biases fused into gate activations.
sigmoid(diag(C M R^T)) replicated on every core at the end.
"""

import os

import numpy as np

N_CORES = 8
B = 64
T = 160
E = 512
H = 1024
V = 32000
S = 2 * B         # 128 sequences (chain A: ctx = cols 0:64, chain B: resp)
BANK_OF = {"f": 0, "i": 1, "g": 2, "o": 3}

# gate order inside the core's 512-wide slice: [f | i | o | g]
OFF = [H, 0, 3 * H, 2 * H]          # rows in Wih/Whh/b for f, i, o, g
GCOL = {"f": 0, "i": 128, "o": 256, "g": 384}
BCOL = {"f": 0, "i": 1, "o": 2, "g": 3}
MM_ORDER = ["f", "i", "g", "o"]     # stop/activation order per step

LAST_EXEC_NS = None
_NC_CACHE = {}


def _build(t_steps=T):
    from contextlib import ExitStack

    import concourse.bacc as bacc
    import concourse.bass as bass
    import concourse.mybir as mybir

    # debug kill-switches for bisection
    no_gather = bool(int(os.environ.get("BASS_KERNEL_NOGATHER", "0")))
    no_bcast = bool(int(os.environ.get("BASS_KERNEL_NOBCAST", "0")))

    f32 = mybir.dt.float32
    bf16 = mybir.dt.bfloat16
    i16 = mybir.dt.int16

    AF = mybir.ActivationFunctionType

    nc = bacc.Bacc(
        "TRN2",
        debug=False,
        num_devices=N_CORES,
        num_swdge_queues=2,
        monotonic_sem_count=N_CORES,
        dynamic_dma_scratch_size=65536,
        detect_race_conditions=not bool(
            int(os.environ.get("BASS_KERNEL_NORACE", "0"))
        ),
    )

    d_tokw = nc.dram_tensor("tokw", [128, 8 * T], i16, kind="ExternalInput")
    d_emb = nc.dram_tensor("emb", [V, E], bf16, kind="ExternalInput")
    d_wihT = nc.dram_tensor("wihT", [8 * E, 512], bf16, kind="ExternalInput")
    d_whhT = nc.dram_tensor("whhT", [8 * H, 512], bf16, kind="ExternalInput")
    d_bias = nc.dram_tensor("biasc", [8 * 128, 4], f32, kind="ExternalInput")
    d_m = nc.dram_tensor("m", [H, H], bf16, kind="ExternalInput")
    d_out = nc.dram_tensor("out", [1, B], f32, kind="ExternalOutput")

    arrs = [nc.monotonic_semaphore(i) for i in range(N_CORES)]

    es = ExitStack()
    sb = lambda name, shape, dt: es.enter_context(nc.sbuf_tensor(name, shape, dt))
    psa = lambda name, shape: es.enter_context(nc.psum_tensor(name, shape, f32))
    sem = lambda name: es.enter_context(nc.semaphore(name))

    tokw = sb("tokw_sb", [128, 8 * T], i16)
    wihT = sb("wihT_sb", [128, 4 * 512], bf16)   # tile (e, m): col e*512 + GCOL[m]
    whhT = sb("whhT_sb", [128, 8 * 512], bf16)   # tile (j, m): col j*512 + GCOL[m]
    bias = sb("bias_sb", [128, 4], f32)          # col BCOL[m]
    msb = sb("msb", [128, 8 * H], bf16)          # row-block i at col i*1024
    # pair-buffer b in {0,1}, e-tile, 256 cols (two steps x two chains):
    # proj(t, c) rhs = xt[:, 4*((t//2)%2)+e, 64*(2*(t%2)+c) : +64]
    xt = sb("xt_sb", [128, 2 * 4, 256], bf16)
    # h^T: chain c, phase ph, slot j at col ((c*2+ph)*8 + j)*64
    hbuf = sb("hbuf", [128, 2 * 2 * 8 * B], bf16)
    # per-chain activation scratch [128, 64]
    sf = [sb(f"sf{c}", [128, B], f32) for c in range(2)]
    si = [sb(f"si{c}", [128, B], f32) for c in range(2)]
    tg = [sb(f"tg{c}", [128, B], f32) for c in range(2)]
    so = [sb(f"so{c}", [128, B], f32) for c in range(2)]
    tc = [sb(f"tc{c}", [128, B], f32) for c in range(2)]
    t1 = [sb(f"t1{c}", [128, B], f32) for c in range(2)]
    t2 = [sb(f"t2{c}", [128, B], f32) for c in range(2)]
    c_sb = [sb(f"c_sb{c}", [128, B], f32) for c in range(2)]
    rh_sb = sb("rh_sb", [128, 8 * B], f32)
    zw_sb = sb("zw_sb", [128, 8 * B], f32)
    ones128 = sb("ones128", [128, 1], f32)
    out_sb = sb("out_sb", [1, B], f32)

    # bank (4*c + BANK_OF[m]): cols bank*512 + 0:64 (single group per bank)
    g_ps = psa("g_ps", [128, 8 * 512])
    z_view = g_ps[:, 0 : 8 * B]         # epilogue reuse: bank 0
    s_view = g_ps[0:1, 512 : 512 + B]   # epilogue reuse: bank 1

    s_ld_tok = sem("s_ld_tok")
    s_ld_wih = sem("s_ld_wih")
    s_ld_bias = sem("s_ld_bias")
    s_ld_whh = sem("s_ld_whh")
    s_ld_m = sem("s_ld_m")
    s_gset = sem("s_gset")
    s_gxp = [sem(f"s_gxp{b}") for b in range(2)]  # per-pair-buffer gather sems
    s_gprep = sem("s_gprep")  # gather descriptors prepared (+1/pair)
    s_prep = sem("s_prep")   # broadcast descriptor prepared (+1/chain-step)
    s_send = sem("s_send")   # broadcast local drain (+16/chain-step)
    s_h = [sem(f"s_h{c}") for c in range(2)]       # DVE wrote own h^T (+1/step)
    s_gates = [sem(f"s_gates{c}") for c in range(2)]  # gate stops (+4/step)
    s_acts = [sem(f"s_acts{c}") for c in range(2)]    # ACT outputs (+4/step)
    s_c = [sem(f"s_c{c}") for c in range(2)]
    s_tc = [sem(f"s_tc{c}") for c in range(2)]
    s_z = sem("s_z")
    s_zmul = sem("s_zmul")
    s_zred = sem("s_zred")
    s_out = sem("s_out")
    s_fin = sem("s_fin")

    pl = (t_steps - 1) % 2  # final h phase

    def hcol(c, ph, j):
        return ((c * 2 + ph) * 8 + j) * B

    with nc.Block() as block:

        # ---------------- SYNC (HWDGE): setup loads + final store ----------
        @block.sync
        def _(sync):
            spid = sync.partition_id()
            sync.dma_start(tokw[:, :], d_tokw[:, :]).then_inc(s_ld_tok, 16)
            for e in range(4):
                sync.dma_start(
                    wihT[:, 512 * e : 512 * (e + 1)],
                    d_wihT[bass.ds(spid * 512 + 128 * e, 128), :],
                ).then_inc(s_ld_wih, 16)
            sync.dma_start(
                bias[:, :], d_bias[bass.ds(spid * 128, 128), :]
            ).then_inc(s_ld_bias, 16)
            for j in range(8):
                sync.dma_start(
                    whhT[:, 512 * j : 512 * (j + 1)],
                    d_whhT[bass.ds(spid * 1024 + 128 * j, 128), :],
                ).then_inc(s_ld_whh, 16)
            for i in range(8):
                sync.dma_start(
                    msb[:, 1024 * i : 1024 * (i + 1)], d_m[128 * i : 128 * (i + 1), :]
                ).then_inc(s_ld_m, 16)
            sync.wait_ge(s_out, 1)
            sync.dma_start(d_out[:, :], out_sb[:, :]).then_inc(s_fin, 16)
            sync.wait_ge(s_fin, 16)

        # ---------------- POOL: gathers + per-sender broadcasts -------------
        @block.gpsimd
        def _(pool):
            pid = pool.partition_id()
            pool.memset(ones128[:, :], 1.0)
            pool.sem_inc(s_gset, 1)
            pool.wait_ge(s_ld_tok, 16)

            n_gprep = [0]

            def gather_pair(p, prepare=False):
                # one transposed gather covers steps 2p and 2p+1 (both chains)
                b = p % 2
                if no_gather:
                    pool.sem_inc(s_gxp[b], 16)
                    return
                if p >= 2:
                    # previous pair on this buffer fully done (race-free count)
                    pool.wait_ge(s_gxp[b], 16 * (p // 2))
                pool.dma_gather(
                    out_ap=xt[:, 4 * b : 4 * (b + 1), :],
                    in_ap=d_emb[:, :],
                    idxs_ap=tokw[:, 16 * p : 16 * (p + 1)],
                    num_idxs=2 * S,
                    num_idxs_reg=2 * S,
                    elem_size=E,
                    transpose=True,
                    prepare_only=prepare,
                    sem=s_gxp[b] if prepare else None,
                    queue_num=1,
                ).then_inc(s_gprep if prepare else s_gxp[b], 1 if prepare else 16)
                if prepare:
                    n_gprep[0] += 1

            gather_pair(0)
            if t_steps > 2:
                gather_pair(1)

            if no_bcast:
                # local-only stand-in: fake all arrivals after own h is ready
                for t in range(t_steps):
                    for c in range(2):
                        pool.wait_ge(s_h[c], t + 1)
                        for k in range(N_CORES):
                            pool.sem_inc(arrs[k].sem(), 2)
                        pool.sem_inc(s_send, 16)
                    if t % 2 == 0 and (t // 2 + 2) * 2 < t_steps:
                        gather_pair(t // 2 + 2)
            else:
                # no loopback slot: own arrival is signaled locally by Pool
                rdests = [None] + [(0, d) for d in range(1, N_CORES)]
                for k in range(N_CORES):
                    with pool.If(pid == k):

                        def prep(n, k=k):
                            t, c = n // 2, n % 2
                            own = hbuf[:, bass.ds(hcol(c, t % 2, k), B)]
                            pool.remote_dma_broadcast(
                                out_ap=own,
                                in_ap=own,
                                remote_sem=arrs[k].sem(),
                                local_sem=s_send,
                                rdests=rdests,
                                queue_num=0,
                            ).then_inc(s_prep, 1)

                        prep(0)
                        n_gprep[0] = 0
                        for t in range(t_steps):
                            for c in range(2):
                                n = 2 * t + c
                                pool.wait_ge(s_prep, n + 1)
                                pool.wait_ge(s_h[c], t + 1)
                                pool.sem_inc(arrs[k].sem(), 2)
                                pool.trigger_dma(count=1, queue_num=0)
                                if n + 1 < 2 * t_steps:
                                    prep(n + 1)
                            if t % 2 == 0 and (t // 2 + 2) * 2 < t_steps:
                                # desc-gen overlaps the flight; fire only after
                                # the broadcasts drained (gather off the wires
                                # during the flight)
                                gather_pair(t // 2 + 2, prepare=True)
                                pool.wait_ge(s_gprep, n_gprep[0])
                                pool.wait_ge(s_send, 16 * (2 * t + 2))
                                pool.trigger_dma(count=1, queue_num=1)

        # ---------------- PE: proj + recurrent matmuls ----------------------
        @block.tensor
        def _(pe):
            def proj(tau, c, first, with_stop=False):
                # gates(tau) x-part for chain c; when not first, each gate
                # waits for ACT's read of gates(tau-1) before re-opening the
                # bank (PE-write/ACT-read same-bank is fatal)
                for mi, m in enumerate(MM_ORDER):
                    gb = (4 * c + BANK_OF[m]) * 512
                    gc = GCOL[m]
                    if not first:
                        pe.wait_ge(s_acts[c], 4 * (tau - 1) + mi + 1)
                    for e in range(4):
                        mm = nc.tensor.matmul(
                            g_ps[:, gb : gb + B],
                            wihT[:, 512 * e + gc : 512 * e + gc + 128],
                            xt[
                                :,
                                4 * ((tau // 2) % 2) + e,
                                64 * (2 * (tau % 2) + c) : 64 * (2 * (tau % 2) + c)
                                + B,
                            ],
                            start=(e == 0),
                            stop=(with_stop and e == 3),
                            skip_group_check=True,
                        )
                        if with_stop and e == 3:
                            mm.then_inc(s_gates[c], 1)

            pe.wait_ge(s_ld_wih, 64)
            pe.wait_ge(s_gxp[0], 16)
            for c in range(2):
                # step 0 has no recurrent part: proj closes the groups
                proj(0, c, first=True, with_stop=True)
            pe.wait_ge(s_ld_whh, 128)

            for t in range(t_steps):
                for c in range(2):
                    if t > 0:
                        q = (t - 1) % 2
                        # per-arrival waits on the first (f) pass
                        for mi, m in enumerate(MM_ORDER):
                            gb = (4 * c + BANK_OF[m]) * 512
                            gc = GCOL[m]
                            for j in range(8):
                                if mi == 0:
                                    pe.wait_ge(arrs[j].sem(), 2 * (2 * t + c - 1))
                                mm = nc.tensor.matmul(
                                    g_ps[:, gb : gb + B],
                                    whhT[:, 512 * j + gc : 512 * j + gc + 128],
                                    hbuf[:, hcol(c, q, j) : hcol(c, q, j) + B],
                                    start=False,
                                    stop=(j == 7),
                                    skip_group_check=True,
                                )
                                if j == 7:
                                    mm.then_inc(s_gates[c], 1)
                    # proj(t+1) fills this chain's broadcast-flight window
                    if t + 1 < t_steps:
                        p_next = (t + 1) // 2
                        pe.wait_ge(s_gxp[p_next % 2], 16 * (p_next // 2 + 1))
                        proj(t + 1, c, first=False)

            # ---------------- bilinear epilogue ----------------
            for j in range(8):
                pe.wait_ge(arrs[j].sem(), 2 * (2 * t_steps))
            for c in range(2):
                pe.wait_ge(s_acts[c], 4 * t_steps)  # banks 0,1 free for reuse
            pe.wait_ge(s_ld_m, 128)
            pe.wait_ge(s_gset, 1)
            for jm in range(8):
                for i in range(8):
                    mm = nc.tensor.matmul(
                        z_view[:, B * jm : B * (jm + 1)],
                        msb[:, 1024 * i + 128 * jm : 1024 * i + 128 * (jm + 1)],
                        hbuf[:, hcol(0, pl, i) : hcol(0, pl, i) + B],
                        start=(i == 0),
                        stop=(i == 7),
                        skip_group_check=True,
                    )
                    if jm == 7 and i == 7:
                        mm.then_inc(s_z, 1)
            pe.wait_ge(s_zmul, 1)
            for jm in range(8):
                mm = nc.tensor.matmul(
                    s_view[:, :],
                    ones128[:, :],
                    zw_sb[:, B * jm : B * (jm + 1)],
                    start=(jm == 0),
                    stop=(jm == 7),
                    skip_group_check=True,
                )
                if jm == 7:
                    mm.then_inc(s_zred, 1)

        # ---------------- ACT (scalar): gate activations --------------------
        @block.scalar
        def _(act):
            act.wait_ge(s_ld_bias, 16)
            for t in range(t_steps):
                for c in range(2):
                    gbase = lambda m: (4 * c + BANK_OF[m]) * 512
                    act.wait_ge(s_gates[c], 4 * t + 1)
                    nc.scalar.activation(
                        sf[c][:, :], g_ps[:, gbase("f") : gbase("f") + B],
                        AF.Sigmoid, bias=bias[:, BCOL["f"] : BCOL["f"] + 1],
                    ).then_inc(s_acts[c], 1)
                    act.wait_ge(s_gates[c], 4 * t + 2)
                    nc.scalar.activation(
                        si[c][:, :], g_ps[:, gbase("i") : gbase("i") + B],
                        AF.Sigmoid, bias=bias[:, BCOL["i"] : BCOL["i"] + 1],
                    ).then_inc(s_acts[c], 1)
                    act.wait_ge(s_gates[c], 4 * t + 3)
                    nc.scalar.activation(
                        tg[c][:, :], g_ps[:, gbase("g") : gbase("g") + B],
                        AF.Tanh, bias=bias[:, BCOL["g"] : BCOL["g"] + 1],
                    ).then_inc(s_acts[c], 1)
                    act.wait_ge(s_gates[c], 4 * t + 4)
                    nc.scalar.activation(
                        so[c][:, :], g_ps[:, gbase("o") : gbase("o") + B],
                        AF.Sigmoid, bias=bias[:, BCOL["o"] : BCOL["o"] + 1],
                    ).then_inc(s_acts[c], 1)
                    act.wait_ge(s_c[c], t + 1)
                    nc.scalar.activation(
                        tc[c][:, :], c_sb[c][:, :], AF.Tanh
                    ).then_inc(s_tc[c], 1)

            act.wait_ge(s_zred, 1)
            nc.scalar.activation(out_sb[:, :], s_view[:, :], AF.Sigmoid).then_inc(
                s_out, 1
            )

        # ---------------- DVE (vector): cell update --------------------------
        @block.vector
        def _(dve):
            dve_pid = dve.partition_id()
            for t in range(t_steps):
                for c in range(2):
                    if t == 0:
                        dve.wait_ge(s_acts[c], 3)
                        nc.vector.tensor_mul(
                            c_sb[c][:, :], si[c][:, :], tg[c][:, :]
                        ).then_inc(s_c[c], 1)
                    else:
                        dve.wait_ge(s_acts[c], 4 * t + 1)
                        nc.vector.tensor_mul(t1[c][:, :], sf[c][:, :], c_sb[c][:, :])
                        dve.wait_ge(s_acts[c], 4 * t + 3)
                        nc.vector.tensor_mul(t2[c][:, :], si[c][:, :], tg[c][:, :])
                        nc.vector.tensor_add(
                            c_sb[c][:, :], t1[c][:, :], t2[c][:, :]
                        ).then_inc(s_c[c], 1)
                    dve.wait_ge(s_tc[c], t + 1)
                    nc.vector.tensor_mul(
                        hbuf[:, bass.ds(hcol(c, t % 2, 0) + dve_pid * B, B)],
                        so[c][:, :],
                        tc[c][:, :],
                    ).then_inc(s_h[c], 1)

            # epilogue: rh cast + elementwise mul
            dve.wait_ge(s_z, 1)
            for jm in range(8):
                nc.vector.tensor_copy(
                    rh_sb[:, B * jm : B * (jm + 1)],
                    hbuf[:, hcol(1, pl, jm) : hcol(1, pl, jm) + B],
                )
            for jm in range(8):
                ins = nc.vector.tensor_mul(
                    zw_sb[:, B * jm : B * (jm + 1)],
                    z_view[:, B * jm : B * (jm + 1)],
                    rh_sb[:, B * jm : B * (jm + 1)],
                )
                if jm == 7:
                    ins.then_inc(s_zmul, 1)

    es.close()
    nc.compile()
    return nc


def _get_nc(t_steps=T):
    if t_steps not in _NC_CACHE:
        _NC_CACHE[t_steps] = _build(t_steps)
    return _NC_CACHE[t_steps]


def _prep_inputs(inputs):
    import ml_dtypes

    bf16 = ml_dtypes.bfloat16
    ctx = np.asarray(inputs["contexts"], np.int64)
    rsp = np.asarray(inputs["responses"], np.int64)
    cat = np.concatenate([ctx, rsp], 0)  # [S, T]
    # wrapped int16 index layout for dma_gather: tokw[p, 8t+j] = cat[16j+p, t],
    # with the 16-partition wrap replicated across all 8 partition groups
    tokw = (
        cat.reshape(8, 16, T).transpose(1, 2, 0).reshape(16, 8 * T).astype(np.int16)
    )
    tokw = np.tile(tokw, (8, 1))  # [128, 8T]

    wih = np.asarray(inputs["Wih"], np.float32)
    whh = np.asarray(inputs["Whh"], np.float32)
    bsum = (
        np.asarray(inputs["bih"], np.float32) + np.asarray(inputs["bhh"], np.float32)
    ).reshape(4 * H)

    # rows[k, m, q] = OFF[m] + 128k + q : core k's gate-slice rows, [f,i,o,g]
    rows = (
        np.asarray(OFF)[None, :, None]
        + np.arange(8)[:, None, None] * 128
        + np.arange(128)[None, None, :]
    )  # [8, 4, 128]
    wihT = wih[rows].transpose(0, 3, 1, 2).reshape(8 * E, 512).astype(bf16)
    whhT = whh[rows].transpose(0, 3, 1, 2).reshape(8 * H, 512).astype(bf16)
    biasc = np.ascontiguousarray(
        bsum[rows].transpose(0, 2, 1).reshape(8 * 128, 4).astype(np.float32)
    )

    return {
        "tokw": np.ascontiguousarray(tokw),
        "emb": np.ascontiguousarray(np.asarray(inputs["emb"], np.float32).astype(bf16)),
        "wihT": np.ascontiguousarray(wihT),
        "whhT": np.ascontiguousarray(whhT),
        "biasc": biasc,
        "m": np.ascontiguousarray(np.asarray(inputs["M"], np.float32).astype(bf16)),
    }


def kernel(**inputs):
    global LAST_EXEC_NS
    from concourse.bass_utils import run_bass_kernel_spmd

    t_steps = int(os.environ.get("BASS_KERNEL_TSTEPS", str(T)))
    nc = _get_nc(t_steps)
    in_map = _prep_inputs(inputs)
    res = run_bass_kernel_spmd(
        nc,
        [dict(in_map) for _ in range(N_CORES)],
        core_ids=list(range(N_CORES)),
        trace=bool(int(os.environ.get("BASS_KERNEL_TRACE", "0"))),
        trace_cores=(
            list(range(N_CORES))
            if int(os.environ.get("BASS_KERNEL_TRACE_ALL", "0"))
            else None
        ),
    )
    LAST_EXEC_NS = res.exec_time_ns
    return res.results[0]["out"].reshape(B).astype(np.float32)


# revision 21
# speedup vs baseline: 1.4051x; 1.4051x over previous
"""DualEncoder (two shared-weight LSTM encoders + bilinear score) on 8 trn2
NeuronCores — transposed-gates redesign.

Reconstructed baseline (1.859 ms, rel err 8.9e-4) — safety-net copy.
"""

import os

import numpy as np

N_CORES = 8
B = 64
T = 160
E = 512
H = 1024
V = 32000
S = 2 * B         # 128 sequences (ctx rows 0:64, resp rows 64:128)
# x^T comes from paired transposed gathers: 2 pair-buffers × (4 e-tiles × 256)
# PSUM: ONE active accumulation group per 2KB bank — each (phase, gate) pair
# gets its own bank: bank = 4*(t%2) + BANK_OF[gate], all 8 banks used.
BANK_OF = {"f": 0, "i": 1, "g": 2, "o": 3}

# gate order inside the core's 512-wide slice: [f | i | o | g]
OFF = [H, 0, 3 * H, 2 * H]          # rows in Wih/Whh/b for f, i, o, g
GCOL = {"f": 0, "i": 128, "o": 256, "g": 384}
BCOL = {"f": 0, "i": 1, "o": 2, "g": 3}
MM_ORDER = ["f", "i", "g", "o"]     # stop/activation order per step

LAST_EXEC_NS = None
_NC_CACHE = {}


def _build(t_steps=T):
    from contextlib import ExitStack

    import concourse.bacc as bacc
    import concourse.bass as bass
    import concourse.mybir as mybir

    # debug kill-switches for bisection
    no_gather = bool(int(os.environ.get("BASS_KERNEL_NOGATHER", "0")))
    no_bcast = bool(int(os.environ.get("BASS_KERNEL_NOBCAST", "0")))

    f32 = mybir.dt.float32
    bf16 = mybir.dt.bfloat16
    i16 = mybir.dt.int16

    AF = mybir.ActivationFunctionType

    nc = bacc.Bacc(
        "TRN2",
        debug=False,
        num_devices=N_CORES,
        num_swdge_queues=2,
        monotonic_sem_count=N_CORES,
        dynamic_dma_scratch_size=65536,
        detect_race_conditions=not bool(
            int(os.environ.get("BASS_KERNEL_NORACE", "0"))
        ),
    )

    d_tokw = nc.dram_tensor("tokw", [128, 8 * T], i16, kind="ExternalInput")
    d_emb = nc.dram_tensor("emb", [V, E], bf16, kind="ExternalInput")
    d_wihT = nc.dram_tensor("wihT", [8 * E, 512], bf16, kind="ExternalInput")
    d_whhT = nc.dram_tensor("whhT", [8 * H, 512], bf16, kind="ExternalInput")
    d_bias = nc.dram_tensor("biasc", [8 * 128, 4], f32, kind="ExternalInput")
    d_m = nc.dram_tensor("m", [H, H], bf16, kind="ExternalInput")
    d_out = nc.dram_tensor("out", [1, B], f32, kind="ExternalOutput")

    arrs = [nc.monotonic_semaphore(i) for i in range(N_CORES)]

    es = ExitStack()
    sb = lambda name, shape, dt: es.enter_context(nc.sbuf_tensor(name, shape, dt))
    psa = lambda name, shape: es.enter_context(nc.psum_tensor(name, shape, f32))
    sem = lambda name: es.enter_context(nc.semaphore(name))

    tokw = sb("tokw_sb", [128, 8 * T], i16)
    wihT = sb("wihT_sb", [128, 4 * 512], bf16)   # tile (e, m): col e*512 + GCOL[m]
    whhT = sb("whhT_sb", [128, 8 * 512], bf16)   # tile (j, m): col j*512 + GCOL[m]
    bias = sb("bias_sb", [128, 4], f32)          # col BCOL[m]
    msb = sb("msb", [128, 8 * H], bf16)          # row-block i at col i*1024
    # pair-buffer b in {0,1}, e-tile, 256 cols (two steps side by side):
    # proj(t) rhs = xt[:, 4*((t//2)%2)+e, 128*(t%2) : 128*(t%2)+128]
    xt = sb("xt_sb", [128, 2 * 4, 256], bf16)
    hbuf = sb("hbuf", [128, 2 * 8 * 128], bf16)  # phase*1024 + slot*128
    sf = sb("sf", [128, 128], f32)
    si = sb("si", [128, 128], f32)
    tg = sb("tg", [128, 128], f32)
    so = sb("so", [128, 128], f32)
    tc = sb("tc", [128, 128], f32)
    t1 = sb("t1", [128, 128], f32)
    t2 = sb("t2", [128, 128], f32)
    c_sb = sb("c_sb", [128, 128], f32)
    rh_sb = sb("rh_sb", [128, 8 * B], f32)
    zw_sb = sb("zw_sb", [128, 8 * B], f32)
    ones128 = sb("ones128", [128, 1], f32)
    out_sb = sb("out_sb", [1, B], f32)

    g_ps = psa("g_ps", [128, 8 * 512])  # bank (4*(t%2)+BANK_OF[m]): cols bank*512+0:128
    z_view = g_ps[:, 0 : 8 * B]         # epilogue reuse: bank 0
    s_view = g_ps[0:1, 512 : 512 + B]   # epilogue reuse: bank 1

    s_ld_tok = sem("s_ld_tok")
    s_ld_wih = sem("s_ld_wih")
    s_ld_bias = sem("s_ld_bias")
    s_ld_whh = sem("s_ld_whh")
    s_ld_m = sem("s_ld_m")
    s_gset = sem("s_gset")
    s_gxp = [sem(f"s_gxp{b}") for b in range(2)]  # per-pair-buffer gather sems
    s_prep = sem("s_prep")   # broadcast descriptor prepared (+1/step)
    s_send = sem("s_send")   # broadcast local drain (+16/step)
    s_h = sem("s_h")         # DVE wrote own h^T slice (+1/step)
    s_gates = sem("s_gates")  # gate tile stops (+4/step, order f,i,g,o)
    s_acts = sem("s_acts")   # ACT outputs (+4/step, order f,i,g,o)
    s_adone = sem("s_adone")  # ACT finished reading bank (+1/step)
    s_c = sem("s_c")
    s_tc = sem("s_tc")
    s_z = sem("s_z")
    s_zmul = sem("s_zmul")
    s_zred = sem("s_zred")
    s_out = sem("s_out")
    s_fin = sem("s_fin")

    pl = (t_steps - 1) % 2  # final h phase

    with nc.Block() as block:

        # ---------------- SYNC (HWDGE): setup loads + final store ----------
        @block.sync
        def _(sync):
            spid = sync.partition_id()
            sync.dma_start(tokw[:, :], d_tokw[:, :]).then_inc(s_ld_tok, 16)
            for e in range(4):
                sync.dma_start(
                    wihT[:, 512 * e : 512 * (e + 1)],
                    d_wihT[bass.ds(spid * 512 + 128 * e, 128), :],
                ).then_inc(s_ld_wih, 16)
            sync.dma_start(
                bias[:, :], d_bias[bass.ds(spid * 128, 128), :]
            ).then_inc(s_ld_bias, 16)
            for j in range(8):
                sync.dma_start(
                    whhT[:, 512 * j : 512 * (j + 1)],
                    d_whhT[bass.ds(spid * 1024 + 128 * j, 128), :],
                ).then_inc(s_ld_whh, 16)
            for i in range(8):
                sync.dma_start(
                    msb[:, 1024 * i : 1024 * (i + 1)], d_m[128 * i : 128 * (i + 1), :]
                ).then_inc(s_ld_m, 16)
            sync.wait_ge(s_out, 1)
            sync.dma_start(d_out[:, :], out_sb[:, :]).then_inc(s_fin, 16)
            sync.wait_ge(s_fin, 16)

        # ---------------- POOL: gathers + per-sender broadcasts -------------
        @block.gpsimd
        def _(pool):
            pid = pool.partition_id()
            pool.memset(ones128[:, :], 1.0)
            pool.sem_inc(s_gset, 1)
            pool.wait_ge(s_ld_tok, 16)

            def gather_pair(p):
                # one transposed gather covers steps 2p and 2p+1
                b = p % 2
                if no_gather:
                    pool.sem_inc(s_gxp[b], 16)
                    return
                if p >= 2:
                    # previous pair on this buffer fully done (race-free count)
                    pool.wait_ge(s_gxp[b], 16 * (p // 2))
                pool.dma_gather(
                    out_ap=xt[:, 4 * b : 4 * (b + 1), :],
                    in_ap=d_emb[:, :],
                    idxs_ap=tokw[:, 16 * p : 16 * (p + 1)],
                    num_idxs=2 * S,
                    num_idxs_reg=2 * S,
                    elem_size=E,
                    transpose=True,
                    queue_num=1,
                ).then_inc(s_gxp[b], 16)

            gather_pair(0)
            if t_steps > 2:
                gather_pair(1)

            if no_bcast:
                # local-only stand-in: fake all arrivals after own h is ready
                for t in range(t_steps):
                    pool.wait_ge(s_h, t + 1)
                    for k in range(N_CORES):
                        pool.sem_inc(arrs[k].sem(), 2)
                    pool.sem_inc(s_send, 16)
                    if t % 2 == 0 and (t // 2 + 2) * 2 < t_steps:
                        gather_pair(t // 2 + 2)
            else:
                # no loopback slot: own arrival is signaled locally by Pool
                rdests = [None] + [(0, d) for d in range(1, N_CORES)]
                for k in range(N_CORES):
                    with pool.If(pid == k):

                        def prep(t, k=k):
                            own = hbuf[:, bass.ds(1024 * (t % 2) + 128 * k, 128)]
                            pool.remote_dma_broadcast(
                                out_ap=own,
                                in_ap=own,
                                remote_sem=arrs[k].sem(),
                                local_sem=s_send,
                                rdests=rdests,
                                queue_num=0,
                            ).then_inc(s_prep, 1)

                        for n0 in range(min(4, t_steps)):
                            prep(n0)
                        for t in range(t_steps):
                            # trigger promptly at h-ready; desc-gen runs 4
                            # steps ahead so worker lib-swap latency never
                            # delays a trigger via s_prep
                            pool.wait_ge(s_prep, t + 1)
                            pool.wait_ge(s_h, t + 1)
                            pool.sem_inc(arrs[k].sem(), 2)
                            pool.trigger_dma(count=1, queue_num=0)
                            if t + 4 < t_steps:
                                prep(t + 4)
                            if t % 2 == 0 and (t // 2 + 2) * 2 < t_steps:
                                # fire gather immediately (desc-gen is async
                                # on the Q7 worker; no queue-blocking waits)
                                gather_pair(t // 2 + 2)

        # ---------------- PE: proj + recurrent matmuls ----------------------
        @block.tensor
        def _(pe):
            def proj(tau, with_stop):
                for m in MM_ORDER:
                    gb = (4 * (tau % 2) + BANK_OF[m]) * 512
                    gc = GCOL[m]
                    for e in range(4):
                        mm = nc.tensor.matmul(
                            g_ps[:, gb : gb + 128],
                            wihT[:, 512 * e + gc : 512 * e + gc + 128],
                            xt[
                                :,
                                4 * ((tau // 2) % 2) + e,
                                128 * (tau % 2) : 128 * (tau % 2) + 128,
                            ],
                            start=(e == 0),
                            stop=(with_stop and e == 3),
                            skip_group_check=True,
                        )
                        if with_stop and e == 3:
                            mm.then_inc(s_gates, 1)

            pe.wait_ge(s_ld_wih, 64)
            pe.wait_ge(s_gxp[0], 16)
            proj(0, with_stop=True)
            if t_steps > 1:
                proj(1, with_stop=False)
            pe.wait_ge(s_ld_whh, 128)

            # keep-warm dummy: N=64 matmul into an idle region of the other
            # phase's f-bank (its groups are closed once ACT(t-1) finished);
            # keeps the PE p-state ramped through the broadcast-flight window
            def dummy(t):
                db = (4 * ((t + 1) % 2)) * 512 + 384
                nc.tensor.matmul(
                    g_ps[:, db : db + 128],
                    wihT[:, 0:128],
                    wihT[:, 0:128],
                    start=True,
                    stop=True,
                    skip_group_check=True,
                )

            KA, KB = 22, 12
            for t in range(1, t_steps):
                q = (t - 1) % 2
                # dummy bank = other phase's f-bank; free once ACT(t-1) read f
                pe.wait_ge(s_acts, 4 * (t - 1) + 1)
                for _ in range(KA):
                    dummy(t)
                pe.wait_ge(s_send, 16 * t)  # own bcast(t-1) drained: mid-flight
                for _ in range(KB):
                    dummy(t)
                # per-arrival waits on the first (f) pass; one accumulation
                # group per (phase, gate) bank
                for mi, m in enumerate(MM_ORDER):
                    gb = (4 * (t % 2) + BANK_OF[m]) * 512
                    gc = GCOL[m]
                    for j in range(8):
                        if mi == 0:
                            pe.wait_ge(arrs[j].sem(), 2 * t)
                        mm = nc.tensor.matmul(
                            g_ps[:, gb : gb + 128],
                            whhT[:, 512 * j + gc : 512 * j + gc + 128],
                            hbuf[:, 1024 * q + 128 * j : 1024 * q + 128 * (j + 1)],
                            start=False,
                            stop=(j == 7),
                            skip_group_check=True,
                        )
                        if j == 7:
                            mm.then_inc(s_gates, 1)
                if t + 1 < t_steps:
                    p_next = (t + 1) // 2
                    pe.wait_ge(s_gxp[p_next % 2], 16 * (p_next // 2 + 1))
                    pe.wait_ge(s_adone, t)  # all other-phase banks free
                    proj(t + 1, with_stop=False)

            # ---------------- bilinear epilogue ----------------
            for j in range(8):
                pe.wait_ge(arrs[j].sem(), 2 * t_steps)
            pe.wait_ge(s_ld_m, 128)
            pe.wait_ge(s_gset, 1)
            for jm in range(8):
                for i in range(8):
                    mm = nc.tensor.matmul(
                        z_view[:, B * jm : B * (jm + 1)],
                        msb[:, 1024 * i + 128 * jm : 1024 * i + 128 * (jm + 1)],
                        hbuf[:, 1024 * pl + 128 * i : 1024 * pl + 128 * i + B],
                        start=(i == 0),
                        stop=(i == 7),
                        skip_group_check=True,
                    )
                    if jm == 7 and i == 7:
                        mm.then_inc(s_z, 1)
            pe.wait_ge(s_zmul, 1)
            for jm in range(8):
                mm = nc.tensor.matmul(
                    s_view[:, :],
                    ones128[:, :],
                    zw_sb[:, B * jm : B * (jm + 1)],
                    start=(jm == 0),
                    stop=(jm == 7),
                    skip_group_check=True,
                )
                if jm == 7:
                    mm.then_inc(s_zred, 1)

        # ---------------- ACT (scalar): gate activations --------------------
        @block.scalar
        def _(act):
            act.wait_ge(s_ld_bias, 16)
            for t in range(t_steps):
                gbase = lambda m: (4 * (t % 2) + BANK_OF[m]) * 512
                act.wait_ge(s_gates, 4 * t + 1)
                nc.scalar.activation(
                    sf[:, :], g_ps[:, gbase("f") : gbase("f") + 128],
                    AF.Sigmoid, bias=bias[:, BCOL["f"] : BCOL["f"] + 1],
                ).then_inc(s_acts, 1)
                act.wait_ge(s_gates, 4 * t + 2)
                nc.scalar.activation(
                    si[:, :], g_ps[:, gbase("i") : gbase("i") + 128],
                    AF.Sigmoid, bias=bias[:, BCOL["i"] : BCOL["i"] + 1],
                ).then_inc(s_acts, 1)
                act.wait_ge(s_gates, 4 * t + 3)
                nc.scalar.activation(
                    tg[:, :], g_ps[:, gbase("g") : gbase("g") + 128],
                    AF.Tanh, bias=bias[:, BCOL["g"] : BCOL["g"] + 1],
                ).then_inc(s_acts, 1)
                act.wait_ge(s_gates, 4 * t + 4)
                nc.scalar.activation(
                    so[:, :], g_ps[:, gbase("o") : gbase("o") + 128],
                    AF.Sigmoid, bias=bias[:, BCOL["o"] : BCOL["o"] + 1],
                ).then_inc(s_acts, 1)
                act.sem_inc(s_adone, 1)
                act.wait_ge(s_c, t + 1)
                nc.scalar.activation(tc[:, :], c_sb[:, :], AF.Tanh).then_inc(s_tc, 1)

            act.wait_ge(s_zred, 1)
            nc.scalar.activation(out_sb[:, :], s_view[:, :], AF.Sigmoid).then_inc(
                s_out, 1
            )

        # ---------------- DVE (vector): cell update --------------------------
        @block.vector
        def _(dve):
            dve_pid = dve.partition_id()
            for t in range(t_steps):
                if t == 0:
                    dve.wait_ge(s_acts, 3)
                    nc.vector.tensor_mul(c_sb[:, :], si[:, :], tg[:, :]).then_inc(
                        s_c, 1
                    )
                else:
                    dve.wait_ge(s_acts, 4 * t + 1)
                    nc.vector.tensor_mul(t1[:, :], sf[:, :], c_sb[:, :])
                    dve.wait_ge(s_acts, 4 * t + 3)
                    nc.vector.tensor_mul(t2[:, :], si[:, :], tg[:, :])
                    nc.vector.tensor_add(c_sb[:, :], t1[:, :], t2[:, :]).then_inc(
                        s_c, 1
                    )
                dve.wait_ge(s_tc, t + 1)
                nc.vector.tensor_mul(
                    hbuf[:, bass.ds(1024 * (t % 2) + dve_pid * 128, 128)],
                    so[:, :],
                    tc[:, :],
                ).then_inc(s_h, 1)

            # epilogue: rh cast + elementwise mul
            dve.wait_ge(s_z, 1)
            for jm in range(8):
                nc.vector.tensor_copy(
                    rh_sb[:, B * jm : B * (jm + 1)],
                    hbuf[:, 1024 * pl + 128 * jm + B : 1024 * pl + 128 * (jm + 1)],
                )
            for jm in range(8):
                ins = nc.vector.tensor_mul(
                    zw_sb[:, B * jm : B * (jm + 1)],
                    z_view[:, B * jm : B * (jm + 1)],
                    rh_sb[:, B * jm : B * (jm + 1)],
                )
                if jm == 7:
                    ins.then_inc(s_zmul, 1)

    es.close()
    nc.compile()
    return nc


def _get_nc(t_steps=T):
    if t_steps not in _NC_CACHE:
        _NC_CACHE[t_steps] = _build(t_steps)
    return _NC_CACHE[t_steps]


def _prep_inputs(inputs):
    import ml_dtypes

    bf16 = ml_dtypes.bfloat16
    ctx = np.asarray(inputs["contexts"], np.int64)
    rsp = np.asarray(inputs["responses"], np.int64)
    cat = np.concatenate([ctx, rsp], 0)  # [S, T]
    # wrapped int16 index layout for dma_gather: tokw[p, 8t+j] = cat[16j+p, t],
    # with the 16-partition wrap replicated across all 8 partition groups
    tokw = (
        cat.reshape(8, 16, T).transpose(1, 2, 0).reshape(16, 8 * T).astype(np.int16)
    )
    tokw = np.tile(tokw, (8, 1))  # [128, 8T]

    wih = np.asarray(inputs["Wih"], np.float32)
    whh = np.asarray(inputs["Whh"], np.float32)
    bsum = (
        np.asarray(inputs["bih"], np.float32) + np.asarray(inputs["bhh"], np.float32)
    ).reshape(4 * H)

    # rows[k, m, q] = OFF[m] + 128k + q : core k's gate-slice rows, [f,i,o,g]
    rows = (
        np.asarray(OFF)[None, :, None]
        + np.arange(8)[:, None, None] * 128
        + np.arange(128)[None, None, :]
    )  # [8, 4, 128]
    wihT = wih[rows].transpose(0, 3, 1, 2).reshape(8 * E, 512).astype(bf16)
    whhT = whh[rows].transpose(0, 3, 1, 2).reshape(8 * H, 512).astype(bf16)
    biasc = np.ascontiguousarray(
        bsum[rows].transpose(0, 2, 1).reshape(8 * 128, 4).astype(np.float32)
    )

    return {
        "tokw": np.ascontiguousarray(tokw),
        "emb": np.ascontiguousarray(np.asarray(inputs["emb"], np.float32).astype(bf16)),
        "wihT": np.ascontiguousarray(wihT),
        "whhT": np.ascontiguousarray(whhT),
        "biasc": biasc,
        "m": np.ascontiguousarray(np.asarray(inputs["M"], np.float32).astype(bf16)),
    }


def kernel(**inputs):
    global LAST_EXEC_NS
    from concourse.bass_utils import run_bass_kernel_spmd

    t_steps = int(os.environ.get("BASS_KERNEL_TSTEPS", str(T)))
    nc = _get_nc(t_steps)
    in_map = _prep_inputs(inputs)
    res = run_bass_kernel_spmd(
        nc,
        [dict(in_map) for _ in range(N_CORES)],
        core_ids=list(range(N_CORES)),
        trace=bool(int(os.environ.get("BASS_KERNEL_TRACE", "0"))),
        trace_cores=(
            list(range(N_CORES))
            if int(os.environ.get("BASS_KERNEL_TRACE_ALL", "0"))
            else None
        ),
    )
    LAST_EXEC_NS = res.exec_time_ns
    return res.results[0]["out"].reshape(B).astype(np.float32)
